# revision 12
# baseline (speedup 1.0000x reference)
"""Trainium2 Bass kernel for the GNN message-passing block (nn_Bind).

Sharding: edges are bucketed by destination-node range (6250 nodes per
core, 8 cores), so the per-destination segment softmax and weighted sum
are fully core-local (no collectives). Within a core, edges are grouped
into 49 windows of 128 destination nodes; each window's edge list is
padded to a multiple of 128 (pad edges carry an all-zero one-hot row, so
they contribute nothing).

Device pipeline per core (all f32 accumulation, bf16 PE operands):
  - K/Q/V projections on TensorE with stationary weights over
    feature-major edge streams (node_emb[src]^T, node_emb[dst]^T, bond^T)
  - per-edge scores: DVE k*q multiply, then a TensorE "bridge" matmul
    against a block-diagonal head mask reduces the 16-dim head groups
  - exp on ScalarE (logits are O(1): segment-max subtraction is a
    mathematical no-op for softmax, so it is skipped)
  - messages v*w, segment-summed into a per-window PSUM accumulator via
    a one-hot matmul (one-hot built on host from the indices)
  - per-window epilogue: beta gating (fused multiply+row-reduce ops),
    LayerNorm (ln-gain/bias folded into W_ff1 on host), FFN, residual,
    final LayerNorm.
"""
import math
import os

import numpy as np
import ml_dtypes

import concourse.bass as bass
import concourse.bacc as bacc
import concourse.mybir as mybir
import concourse.tile as tile
from concourse.bass_utils import run_bass_kernel_spmd

BF = ml_dtypes.bfloat16
F32 = np.float32

N, D, H = 50000, 128, 8
HD = D // H            # 16
NCORES = 8
NPC = N // NCORES      # 6250 nodes per core
P = 128
W = (NPC + P - 1) // P  # 49 windows per core

bf16 = mybir.dt.bfloat16
fp32 = mybir.dt.float32
AF = mybir.ActivationFunctionType
ALU = mybir.AluOpType


def _bcast(ap, dims):
    """Manual AP with explicit [step, count] dims (for stride-0 broadcasts)."""
    return bass.AP(ap.tensor, ap.offset, [list(x) for x in dims])


def _prep(node_emb, bond_emb, basic_attn, src, dst, Wk, Wq, Wv, W_dis,
          W_beta, ln1_g, ln1_b, W_ff1, W_ff2):
    """Host-side sharding: bucket/sort edges by destination, build per-core
    padded feature-major streams and one-hot segment matrices."""
    E = src.shape[0]
    src = src.astype(np.int64)
    dst = dst.astype(np.int64)

    core = dst // NPC
    local = dst - core * NPC
    wloc = local // P
    slot = local % P
    key = core * W + wloc
    order = np.argsort(key, kind="stable")

    counts = np.bincount(key, minlength=NCORES * W).reshape(NCORES, W)
    K_w = (counts.max(axis=0) + P - 1) // P          # chunks per window
    K_w = np.maximum(K_w, 1).astype(np.int64)
    cap_w = K_w * P
    off_w = np.concatenate([[0], np.cumsum(cap_w)]).astype(np.int64)
    E_pad = int(off_w[-1])

    group_start = np.zeros(NCORES * W, np.int64)
    group_start[1:] = np.cumsum(counts.reshape(-1))[:-1]
    pos = np.arange(E) - group_start[key[order]]
    eslot = off_w[wloc[order]] + pos

    # host-side weight prep (weights only -- data arrays are just laid out)
    wa = (W_beta[0:D, 0] + W_beta[2 * D:3 * D, 0]).astype(F32)
    wb = (W_beta[D:2 * D, 0] - W_beta[2 * D:3 * D, 0]).astype(F32)
    W1p = (ln1_g[:, None] * W_ff1).astype(F32)        # [128,256]
    bias1 = (ln1_b.astype(F32) @ W_ff1.astype(F32))   # [256]

    consts = {
        "wk": np.ascontiguousarray(Wk, dtype=BF),
        "wq": np.ascontiguousarray(Wq, dtype=BF),
        "wv": np.ascontiguousarray(Wv, dtype=BF),
        "bdm": np.ascontiguousarray(
            (np.arange(D)[:, None] // HD == np.arange(H)[None, :]), dtype=BF),
        "wdis4": np.ascontiguousarray(
            np.tile(4.0 * W_dis.reshape(1, H).astype(F32), (P, 1))),
        "warep": np.ascontiguousarray(np.tile(wa[None, :], (P, 1))),
        "wbrep": np.ascontiguousarray(np.tile(wb[None, :], (P, 1))),
        "w1p": np.ascontiguousarray(W1p, dtype=BF),
        "b1": np.ascontiguousarray(bias1.reshape(2, P).T.astype(F32)),
        "w2t": np.ascontiguousarray(
            W_ff2.reshape(2, P, D).transpose(1, 0, 2).reshape(P, 2 * D), dtype=BF),
        "ident": np.ascontiguousarray(np.eye(P), dtype=BF),
    }

    in_maps = []
    core_sorted = core[order]
    for c in range(NCORES):
        m = core_sorted == c
        es = eslot[m]
        e_ids = order[m]
        srcT = np.zeros((D, E_pad), BF)
        dstT = np.zeros((D, E_pad), BF)
        bondT = np.zeros((D, E_pad), BF)
        srcT[:, es] = node_emb[src[e_ids]].T
        dstT[:, es] = node_emb[dst[e_ids]].T
        bondT[:, es] = bond_emb[e_ids].T
        battn = np.zeros((P, E_pad // P), F32)
        battn[es % P, es // P] = basic_attn[e_ids]
        oh = np.zeros((P, E_pad), BF)
        oh[es % P, (es // P) * P + slot[e_ids]] = 1.0
        x = np.zeros((P, W, D), F32)
        xsrc = node_emb[c * NPC:(c + 1) * NPC].reshape(-1, D)
        wfull = NPC // P
        x[:, :wfull, :] = xsrc[:wfull * P].reshape(wfull, P, D).transpose(1, 0, 2)
        rem = NPC - wfull * P
        if rem:
            x[:rem, wfull, :] = xsrc[wfull * P:]
        im = {
            "srcT": srcT, "dstT": dstT, "bondT": bondT,
            "battn": battn, "oh": oh,
            "x": np.ascontiguousarray(x.reshape(P, W * D)),
        }
        im.update(consts)
        in_maps.append(im)

    return in_maps, K_w.tolist(), E_pad


def _build(K_w, E_pad):
    nc = bacc.Bacc(None, target_bir_lowering=False)
    NCHUNK = E_pad // P

    srcT = nc.dram_tensor("srcT", [P, E_pad], bf16, kind="ExternalInput")
    dstT = nc.dram_tensor("dstT", [P, E_pad], bf16, kind="ExternalInput")
    bondT = nc.dram_tensor("bondT", [P, E_pad], bf16, kind="ExternalInput")
    battn = nc.dram_tensor("battn", [P, NCHUNK], fp32, kind="ExternalInput")
    ohd = nc.dram_tensor("oh", [P, E_pad], bf16, kind="ExternalInput")
    xd = nc.dram_tensor("x", [P, W * D], fp32, kind="ExternalInput")
    wkd = nc.dram_tensor("wk", [P, D], bf16, kind="ExternalInput")
    wqd = nc.dram_tensor("wq", [P, D], bf16, kind="ExternalInput")
    wvd = nc.dram_tensor("wv", [P, D], bf16, kind="ExternalInput")
    bdmd = nc.dram_tensor("bdm", [P, H], bf16, kind="ExternalInput")
    wdis4d = nc.dram_tensor("wdis4", [P, H], fp32, kind="ExternalInput")
    warepd = nc.dram_tensor("warep", [P, D], fp32, kind="ExternalInput")
    wbrepd = nc.dram_tensor("wbrep", [P, D], fp32, kind="ExternalInput")
    w1pd = nc.dram_tensor("w1p", [P, 2 * D], bf16, kind="ExternalInput")
    b1d = nc.dram_tensor("b1", [P, 2], fp32, kind="ExternalInput")
    w2td = nc.dram_tensor("w2t", [P, 2 * D], bf16, kind="ExternalInput")
    identd = nc.dram_tensor("ident", [P, P], bf16, kind="ExternalInput")
    outd = nc.dram_tensor("out", [P, W * D], fp32, kind="ExternalOutput")

    woff = np.concatenate([[0], np.cumsum(K_w)]).astype(int)  # chunk offsets

    with tile.TileContext(nc) as tc:
        with (
            tc.tile_pool(name="const", bufs=1) as cpool,
            tc.tile_pool(name="stream", bufs=3) as spool,
            tc.tile_pool(name="mid", bufs=3) as mpool,
            tc.tile_pool(name="epi", bufs=2) as epool,
            tc.tile_pool(name="small", bufs=3) as tpool,
            tc.tile_pool(name="psA", bufs=2, space="PSUM") as psA,
            tc.tile_pool(name="psB", bufs=1, space="PSUM") as psB,
            tc.tile_pool(name="psft", bufs=1, space="PSUM") as psft,
        ):
            def cload(dram, shape, dtype, tag):
                t = cpool.tile(shape, dtype, tag=tag)
                nc.sync.dma_start(out=t[:], in_=dram[:])
                return t

            wk_sb = cload(wkd, [P, D], bf16, "c_wk")
            wq_sb = cload(wqd, [P, D], bf16, "c_wq")
            wv_sb = cload(wvd, [P, D], bf16, "c_wv")
            bdm_sb = cload(bdmd, [P, H], bf16, "c_bdm")
            wdis4_sb = cload(wdis4d, [P, H], fp32, "c_wdis")
            warep_sb = cload(warepd, [P, D], fp32, "c_wa")
            wbrep_sb = cload(wbrepd, [P, D], fp32, "c_wb")
            w1p_sb = cload(w1pd, [P, 2 * D], bf16, "c_w1p")
            b1_sb = cload(b1d, [P, 2], fp32, "c_b1")
            w2t_sb = cload(w2td, [P, 2 * D], bf16, "c_w2t")
            ident_sb = cload(identd, [P, P], bf16, "c_ident")
            battn_sb = cload(battn, [P, NCHUNK], fp32, "c_battn")
            xall_sb = cload(xd, [P, W * D], fp32, "c_x")
            outall_sb = cpool.tile([P, W * D], fp32)
            eps_sb = cpool.tile([P, 1], fp32)
            nc.vector.memset(eps_sb[:], 1e-5)

            # distance-decay logits for all edges: battn * (4*W_dis), [P, NCHUNK, H]
            dist_all = cpool.tile([P, NCHUNK, H], fp32, tag="c_dist")
            nc.vector.tensor_mul(
                dist_all[:],
                _bcast(battn_sb[:], [[NCHUNK, P], [1, NCHUNK], [0, H]]),
                _bcast(wdis4_sb[:], [[H, P], [0, NCHUNK], [1, H]]),
            )

            for w in range(W):
                kw = K_w[w]
                c0 = woff[w]
                ft = psft.tile([P, 136], fp32, tag="ft")

                # tiles of up to 4 chunks (512 edges)
                tsizes = []
                rem = kw
                while rem > 0:
                    t = min(4, rem)
                    tsizes.append(t)
                    rem -= t
                t0 = 0
                for nt in tsizes:
                    Et = nt * P
                    ecol = (c0 + t0) * P
                    src_t = spool.tile([P, Et], bf16, tag="src")
                    nc.sync.dma_start(out=src_t[:], in_=srcT[:, ecol:ecol + Et])
                    dst_t = spool.tile([P, Et], bf16, tag="dst")
                    nc.sync.dma_start(out=dst_t[:], in_=dstT[:, ecol:ecol + Et])
                    bond_t = spool.tile([P, Et], bf16, tag="bond")
                    nc.sync.dma_start(out=bond_t[:], in_=bondT[:, ecol:ecol + Et])
                    oh_t = spool.tile([P, Et], bf16, tag="oh")
                    nc.sync.dma_start(out=oh_t[:], in_=ohd[:, ecol:ecol + Et])

                    k_ps = psA.tile([P, Et], fp32, tag="k")
                    nc.tensor.matmul(k_ps[:], lhsT=wk_sb[:], rhs=src_t[:],
                                     start=True, stop=True)
                    q_ps = psA.tile([P, Et], fp32, tag="q")
                    nc.tensor.matmul(q_ps[:], lhsT=wq_sb[:], rhs=dst_t[:],
                                     start=True, stop=True)
                    q_sb = mpool.tile([P, Et], fp32, tag="qsb")
                    nc.scalar.copy(q_sb[:], q_ps[:])
                    kq_t = mpool.tile([P, Et], bf16, tag="kq")
                    nc.vector.tensor_mul(kq_t[:], k_ps[:], q_sb[:])

                    sc_ps = psB.tile([P, nt, H], fp32, tag="sc")
                    for c in range(nt):
                        nc.tensor.matmul(sc_ps[:, c, :],
                                         lhsT=kq_t[:, c * P:(c + 1) * P],
                                         rhs=bdm_sb[:], start=True, stop=True)
                    sc2 = mpool.tile([P, nt, H], fp32, tag="sc2")
                    nc.vector.tensor_add(sc2[:], sc_ps[:],
                                         dist_all[:, c0 + t0:c0 + t0 + nt, :])

                    v_ps = psB.tile([P, Et], fp32, tag="v")
                    for c in range(nt):
                        nc.tensor.matmul(v_ps[:, c * P:(c + 1) * P],
                                         lhsT=bond_t[:, c * P:(c + 1) * P],
                                         rhs=wv_sb[:], start=True, stop=True)

                    msg_t = mpool.tile([P, nt, 136], bf16, tag="msg")
                    wexp = mpool.tile([P, nt, H], fp32, tag="wexp")
                    nc.scalar.activation(wexp[:], sc2[:], AF.Exp, scale=0.25)
                    nc.scalar.activation(msg_t[:, :, D:], sc2[:], AF.Exp,
                                         scale=0.25)
                    nc.vector.tensor_mul(
                        msg_t[:, :, 0:D].rearrange("p c (h e) -> p c h e", h=H),
                        v_ps[:].rearrange("p (c h e) -> p c h e", c=nt, h=H),
                        wexp[:].to_broadcast([P, nt, H, HD]),
                    )

                    for c in range(nt):
                        nc.tensor.matmul(
                            ft[:],
                            lhsT=oh_t[:, c * P:(c + 1) * P],
                            rhs=msg_t[:, c, :],
                            start=(t0 == 0 and c == 0),
                            stop=(t0 + nt == kw and c == nt - 1),
                        )
                    t0 += nt

                # ---- epilogue for window w ----
                STAGE = int(os.environ.get("KSTAGE", "0"))
                xw = xall_sb[:, w * D:(w + 1) * D]

                den = tpool.tile([P, H], fp32, tag="den")
                nc.vector.tensor_scalar_add(den[:], ft[:, D:], 1e-16)
                invd = tpool.tile([P, H], fp32, tag="invd")
                nc.vector.reciprocal(invd[:], den[:])

                he = epool.tile([P, D], fp32, tag="he")
                nc.vector.scalar_tensor_tensor(
                    out=he[:].rearrange("p (h e) -> p h e", h=H),
                    in0=ft[:, 0:D].rearrange("p (h e) -> p h e", h=H),
                    scalar=1.0, op0=ALU.bypass,
                    in1=invd[:].to_broadcast([P, H, HD]),
                    op1=ALU.mult,
                )

                if STAGE == 1:
                    nc.vector.tensor_copy(outall_sb[:, w * D:(w + 1) * D], he[:])
                    continue
                zjunk = epool.tile([P, D], fp32, tag="zjunk")
                z1 = tpool.tile([P, 1], fp32, tag="z1")
                z2 = tpool.tile([P, 1], fp32, tag="z2")
                # (tensor_tensor_reduce faults on this runtime; use mul+reduce)
                nc.vector.tensor_mul(zjunk[:], he[:], warep_sb[:])
                nc.vector.reduce_sum(z1[:], zjunk[:], axis=mybir.AxisListType.X)
                zjunk2 = epool.tile([P, D], fp32, tag="zjunk2")
                nc.vector.tensor_mul(zjunk2[:], xw, wbrep_sb[:])
                zb = tpool.tile([P, 1], fp32, tag="zb")
                nc.vector.reduce_sum(zb[:], zjunk2[:], axis=mybir.AxisListType.X)
                nc.vector.tensor_add(z2[:], z1[:], zb[:])
                beta = tpool.tile([P, 1], fp32, tag="beta")
                nc.scalar.activation(beta[:], z2[:], AF.Sigmoid)

                tdif = epool.tile([P, D], fp32, tag="tdif")
                nc.vector.tensor_sub(tdif[:], xw, he[:])
                he2 = epool.tile([P, D], fp32, tag="he2")
                nc.vector.scalar_tensor_tensor(
                    out=he2[:], in0=tdif[:], scalar=beta[:], op0=ALU.mult,
                    in1=he[:], op1=ALU.add)

                if STAGE == 2:
                    nc.vector.tensor_copy(outall_sb[:, w * D:(w + 1) * D], he2[:])
                    continue
                # LayerNorm 1 (gain/bias folded into w1p/b1)
                musum = tpool.tile([P, 1], fp32, tag="musum")
                junkbf = epool.tile([P, D], bf16, tag="junkbf")
                nc.scalar.activation(junkbf[:], he2[:], AF.Copy,
                                     accum_out=musum[:])
                negmu = tpool.tile([P, 1], fp32, tag="negmu")
                nc.vector.tensor_scalar_mul(negmu[:], musum[:], -1.0 / D)
                sqj = epool.tile([P, D], fp32, tag="sqj")
                varsum = tpool.tile([P, 1], fp32, tag="varsum")
                nc.scalar.activation(sqj[:], he2[:], AF.Square, bias=negmu[:],
                                     accum_out=varsum[:])
                std = tpool.tile([P, 1], fp32, tag="std")
                nc.scalar.activation(std[:], varsum[:], AF.Sqrt, scale=1.0 / D,
                                     bias=eps_sb[:])
                rstd = tpool.tile([P, 1], fp32, tag="rstd")
                nc.vector.reciprocal(rstd[:], std[:])
                nmr = tpool.tile([P, 1], fp32, tag="nmr")
                nc.vector.tensor_mul(nmr[:], negmu[:], rstd[:])
                hhat = epool.tile([P, D], bf16, tag="hhat")
                nc.scalar.activation(hhat[:], he2[:], AF.Identity,
                                     scale=rstd[:], bias=nmr[:])

                # FFN in transposed (feature-major) layout
                ht_ps = psB.tile([P, P], bf16, tag="epps")
                nc.tensor.transpose(ht_ps[:], hhat[:], ident_sb[:])
                ht = epool.tile([P, P], bf16, tag="ht")
                nc.scalar.copy(ht[:], ht_ps[:])

                hid_ps = psB.tile([P, 2, P], fp32, tag="epps")
                nc.tensor.matmul(hid_ps[:, 0, :], lhsT=w1p_sb[:, 0:P],
                                 rhs=ht[:], start=True, stop=True)
                nc.tensor.matmul(hid_ps[:, 1, :], lhsT=w1p_sb[:, P:2 * P],
                                 rhs=ht[:], start=True, stop=True)
                relu_t = epool.tile([P, 2, P], bf16, tag="relu")
                nc.scalar.activation(relu_t[:, 0, :], hid_ps[:, 0, :], AF.Relu,
                                     bias=b1_sb[:, 0:1])
                nc.scalar.activation(relu_t[:, 1, :], hid_ps[:, 1, :], AF.Relu,
                                     bias=b1_sb[:, 1:2])

                o2t_ps = psB.tile([P, P], fp32, tag="epps")
                nc.tensor.matmul(o2t_ps[:], lhsT=w2t_sb[:, 0:P],
                                 rhs=relu_t[:, 0, :], start=True, stop=False)
                nc.tensor.matmul(o2t_ps[:], lhsT=w2t_sb[:, P:2 * P],
                                 rhs=relu_t[:, 1, :], start=False, stop=True)
                o2bf = epool.tile([P, P], bf16, tag="o2bf")
                nc.scalar.copy(o2bf[:], o2t_ps[:])
                o2_ps = psB.tile([P, P], bf16, tag="epps")
                nc.tensor.transpose(o2_ps[:], o2bf[:], ident_sb[:])

                res = epool.tile([P, D], fp32, tag="res")
                nc.vector.tensor_add(res[:], o2_ps[:], he2[:])
                if STAGE == 3:
                    nc.vector.tensor_copy(outall_sb[:, w * D:(w + 1) * D], res[:])
                    continue

                # final LayerNorm (unit gain, zero bias)
                musum2 = tpool.tile([P, 1], fp32, tag="musum2")
                junkbf2 = epool.tile([P, D], bf16, tag="junkbf2")
                nc.scalar.activation(junkbf2[:], res[:], AF.Copy,
                                     accum_out=musum2[:])
                negmu2 = tpool.tile([P, 1], fp32, tag="negmu2")
                nc.vector.tensor_scalar_mul(negmu2[:], musum2[:], -1.0 / D)
                sqj2 = epool.tile([P, D], fp32, tag="sqj2")
                varsum2 = tpool.tile([P, 1], fp32, tag="varsum2")
                nc.scalar.activation(sqj2[:], res[:], AF.Square, bias=negmu2[:],
                                     accum_out=varsum2[:])
                std2 = tpool.tile([P, 1], fp32, tag="std2")
                nc.scalar.activation(std2[:], varsum2[:], AF.Sqrt,
                                     scale=1.0 / D, bias=eps_sb[:])
                rstd2 = tpool.tile([P, 1], fp32, tag="rstd2")
                nc.vector.reciprocal(rstd2[:], std2[:])
                nmr2 = tpool.tile([P, 1], fp32, tag="nmr2")
                nc.vector.tensor_mul(nmr2[:], negmu2[:], rstd2[:])
                nc.scalar.activation(outall_sb[:, w * D:(w + 1) * D], res[:],
                                     AF.Identity, scale=rstd2[:], bias=nmr2[:])

            nc.sync.dma_start(out=outd[:], in_=outall_sb[:])
    nc.finalize()
    return nc


def kernel(**inputs):
    args = {k: np.asarray(v) for k, v in inputs.items()}
    in_maps, K_w, E_pad = _prep(
        node_emb=args["node_emb"].astype(F32),
        bond_emb=args["bond_emb"].astype(F32),
        basic_attn=args["basic_attn"].astype(F32),
        src=args["src"], dst=args["dst"],
        Wk=args["Wk"].astype(F32), Wq=args["Wq"].astype(F32),
        Wv=args["Wv"].astype(F32), W_dis=args["W_dis"].astype(F32),
        W_beta=args["W_beta"].astype(F32),
        ln1_g=args["ln1_g"].astype(F32), ln1_b=args["ln1_b"].astype(F32),
        W_ff1=args["W_ff1"].astype(F32), W_ff2=args["W_ff2"].astype(F32),
    )
    nc = _build(K_w, E_pad)
    res = run_bass_kernel_spmd(nc, in_maps, list(range(NCORES)),
                               trace=bool(int(os.environ.get("KTRACE", "0"))))
    global LAST_RESULT
    LAST_RESULT = res
    out = np.empty((N, D), F32)
    for c in range(NCORES):
        oc = np.asarray(res.results[c]["out"], F32).reshape(P, W, D)
        oc = oc.transpose(1, 0, 2).reshape(W * P, D)
        out[c * NPC:(c + 1) * NPC] = oc[:NPC]
    return out


LAST_RESULT = None
